# revision 2
# baseline (speedup 1.0000x reference)
"""ROIPooling (adaptive 7x7 max over per-ROI valid h x w) on 8 trn2 cores.

v3 strategy ("V-select on DMA, V-merge + H on DVE/ACT", all bf16):
  - Host re-lays x per core as xg[(roi,chh,r), (w,c)] bf16 rows of 3584B and
    computes, from h, per-(partition,bin) candidate row indices:
      A = s_b, B = s_b+1 (dup s_b if width<2), C = e_b-1 (dup if width<3;
      skipped for b=0 where width<=2 always).
    Duplicates are harmless under max, so the data-dependent VERTICAL row
    selection is ONE indirect-DMA gather per b-chunk (runs of (w,c) planes,
    landing [p][slot][w,c]) -- bulk DMA bandwidth, no masks.  (The DMA
    cce max-landing path is rejected by walrus, so the candidate layers land
    flat and are max-merged by 2 bf16 tensor_tensor ops per chunk on DVE.)
  - HORIZONTAL pooling is mask-based:
      leaves: per-(p,slot) additive mask {0,-3e38} via tensor_scalar (bf16 4x
      mode, 0.26 ns/elem) on DVE, with a large share on the otherwise-idle
      ACT engine (activation Identity + bias); combines: 7 rr-diagonal
      tensor_tensor maxes (bf16 2x) on DVE.
  - b-chunked so H compute overlaps later gathers; bf16 halves DMA bytes
    (rel err ~2^-9, far under the 2e-2 gate).
"""

import numpy as np
import ml_dtypes
from contextlib import ExitStack

import concourse.bass as bass
import concourse.bacc as bacc
import concourse.tile as tile
from concourse import mybir
from concourse.bass import IndirectOffsetOnAxis
from concourse.bass_utils import run_bass_kernel_spmd

N, C, H, W, OUT = 512, 256, 14, 14, 7
NCORES = 8
NS = N // NCORES          # ROIs per core
CH = C // 2               # channels per partition (2 partitions per ROI)
NEG = -3.0e38
WPAD = [2, 3, 4, 5, 6, 7, 8]
# H-stage slot list, rr-major ragged:  (j, rr) with rr < WPAD[j]
SLOTS = [(j, rr) for rr in range(8) for j in range(OUT) if rr < WPAD[j]]
RR0 = [(i, jr) for i, jr in enumerate(SLOTS) if jr[1] == 0]
RRK = [(i, jr) for i, jr in enumerate(SLOTS) if jr[1] >= 1]
SIDX = {i: k for k, (i, _) in enumerate(RRK)}
CHUNKS = [(0, 2), (2, 2), (4, 3)]    # (b0, bsl)
ACT_LEAVES = [24, 24, 18]            # per-chunk leaves offloaded to ACT

FP32 = mybir.dt.float32
BF16 = mybir.dt.bfloat16
I32 = mybir.dt.int32
BF = ml_dtypes.bfloat16


def chunk_gslots(b0, bsl):
    """gather slot list for a chunk: layers A,B all bins; C skips b=0."""
    bs = list(range(b0, b0 + bsl))
    return ([(0, b) for b in bs] + [(1, b) for b in bs]
            + [(2, b) for b in bs if b != 0])


GIDX_OF = {}          # (lay, b) -> column in the idx tensor
_gall = []
for _b0, _bsl in CHUNKS:
    for _lb in chunk_gslots(_b0, _bsl):
        GIDX_OF[_lb] = len(_gall)
        _gall.append(_lb)
NGI = len(_gall)


def _bins(L):
    i = np.arange(OUT)
    s = (i * L) // OUT
    e = ((i + 1) * L + OUT - 1) // OUT
    return s, e


def build_program():
    nc = bacc.Bacc("TRN2", target_bir_lowering=False, debug=False,
                   num_devices=NCORES)
    xg = nc.dram_tensor("xg", [128 * H, W * CH], BF16,
                        kind="ExternalInput").ap()
    idx = nc.dram_tensor("idx", [128, NGI], I32, kind="ExternalInput").ap()
    hm = nc.dram_tensor("hm", [128, len(SLOTS)], FP32,
                        kind="ExternalInput").ap()
    out = nc.dram_tensor("out", [128, OUT, OUT, CH], BF16,
                         kind="ExternalOutput").ap()

    ADD = mybir.AluOpType.add
    MAX = mybir.AluOpType.max
    IDENT = mybir.ActivationFunctionType.Identity

    with tile.TileContext(nc) as tc, ExitStack() as ctx:
        singles = ctx.enter_context(tc.tile_pool(name="singles", bufs=1))
        big = ctx.enter_context(tc.tile_pool(name="big", bufs=1))

        # warm the ACT table during the first gather
        warm = singles.tile([128, 1], BF16, name="warm")
        nc.vector.memset(warm, 0.0)
        nc.scalar.activation(out=warm, in_=warm, func=IDENT,
                             bias=0.0, scale=1.0)

        idx_t = singles.tile([128, NGI], I32)
        nc.sync.dma_start(idx_t[:], idx)
        hm_t = singles.tile([128, len(SLOTS)], FP32)
        nc.sync.dma_start(hm_t[:], hm)

        T, S, O, goff = {}, {}, {}, {}
        off = 0
        for b0, bsl in CHUNKS:
            ns = len(chunk_gslots(b0, bsl))
            T[b0] = big.tile([128, ns, W * CH], BF16, tag=f"T{b0}",
                             name=f"T{b0}")
            S[b0] = big.tile([128, len(RRK), bsl, CH], BF16, tag=f"S{b0}",
                             name=f"S{b0}")
            O[b0] = big.tile([128, OUT, bsl, CH], BF16, tag=f"O{b0}",
                             name=f"O{b0}")
            goff[b0] = off
            off += ns

        # one gather per slot ([P,1] idx = the HW-supported indirect form)
        for b0, bsl in CHUNKS:
            for si in range(len(chunk_gslots(b0, bsl))):
                col = goff[b0] + si
                nc.gpsimd.indirect_dma_start(
                    out=T[b0][:, si], out_offset=None, in_=xg,
                    in_offset=IndirectOffsetOnAxis(
                        ap=idx_t[:, col:col + 1], axis=0))

        for ci, (b0, bsl) in enumerate(CHUNKS):
            act_budget = ACT_LEAVES[ci]
            ns = len(chunk_gslots(b0, bsl))
            ncb = ns - 2 * bsl              # number of C slots (bsl or bsl-1)
            # V-merge: A |= B (all bins), then |= C (C bins)
            A = T[b0][:, 0:bsl]
            Bv = T[b0][:, bsl:2 * bsl]
            nc.vector.tensor_tensor(out=A, in0=A, in1=Bv, op=MAX)
            if ncb > 0:
                Ac = T[b0][:, bsl - ncb:bsl]
                Cv = T[b0][:, 2 * bsl:2 * bsl + ncb]
                nc.vector.tensor_tensor(out=Ac, in0=Ac, in1=Cv, op=MAX)

            # H leaves (masked w-slices of merged A); share to ACT
            nact = 0
            for k, (i, (j, rr)) in enumerate(RRK):
                src = T[b0][:, 0:bsl, (j + rr) * CH:(j + rr + 1) * CH]
                dst = S[b0][:, SIDX[i]]
                msk = hm_t[:, i:i + 1]
                if nact < act_budget and (k * act_budget) // len(RRK) >= nact:
                    nc.scalar.activation(out=dst, in_=src, func=IDENT,
                                         bias=msk, scale=1.0)
                    nact += 1
                else:
                    nc.vector.tensor_scalar(out=dst, in0=src, scalar1=msk,
                                            scalar2=None, op0=ADD)
            for i, (j, rr) in RR0:
                nc.vector.tensor_scalar(
                    out=O[b0][:, j],
                    in0=T[b0][:, 0:bsl, j * CH:(j + 1) * CH],
                    scalar1=hm_t[:, i:i + 1], scalar2=None, op0=ADD)
            # combines: one TT max per rr diagonal
            soff = 0
            for rr in range(1, 8):
                jlo = max(0, rr - 1)
                n = OUT - jlo
                nc.vector.tensor_tensor(
                    out=O[b0][:, jlo:], in0=O[b0][:, jlo:],
                    in1=S[b0][:, soff:soff + n], op=MAX)
                soff += n
            nc.sync.dma_start(out[:, :, b0:b0 + bsl], O[b0][:])

        del T, S, O

    nc.compile()
    return nc


def make_in_maps(rois, h, w):
    rois = np.ascontiguousarray(rois, np.float32).reshape(N, C, H, W)
    h = np.asarray(h).astype(np.int64)
    w = np.asarray(w).astype(np.int64)
    in_maps = []
    jj = np.arange(OUT)
    for k in range(NCORES):
        sl = slice(k * NS, (k + 1) * NS)
        # [64, 2, 128, 14, 14] -> [(roi,chh,r), (w,c)]
        xk = rois[sl].reshape(NS, 2, CH, H, W).transpose(0, 1, 3, 4, 2)
        xg = np.ascontiguousarray(xk, dtype=BF).reshape(128 * H, W * CH)

        idx = np.zeros((128, NGI), np.int32)
        hmm = np.zeros((128, len(SLOTS)), np.float32)
        hk, wk = h[sl], w[sl]
        for nroi in range(NS):
            s, e = _bins(int(hk[nroi]))
            wid = e - s
            rows = (s, np.where(wid >= 2, s + 1, s), e - 1)
            for chh in range(2):
                p = 2 * nroi + chh
                base = p * H
                for (lay, b), col in GIDX_OF.items():
                    idx[p, col] = base + rows[lay][b]
            sw, ew = _bins(int(wk[nroi]))
            m = np.where((jj[:, None] + np.arange(8)[None, :] >= sw[:, None])
                         & (jj[:, None] + np.arange(8)[None, :] < ew[:, None]),
                         0.0, NEG).astype(np.float32)
            mv = np.array([m[j, rr] for (j, rr) in SLOTS], np.float32)
            hmm[2 * nroi] = mv
            hmm[2 * nroi + 1] = mv
        in_maps.append({"xg": xg, "idx": idx, "hm": hmm})
    return in_maps


def unpack_out(res):
    outs = []
    for k in range(NCORES):
        o = np.asarray(res.results[k]["out"]).astype(np.float32)
        # [128(p=(roi,chh)), j, i, c] -> [roi, (chh,c), i, j]
        o = o.reshape(NS, 2, OUT, OUT, CH).transpose(0, 1, 4, 3, 2)
        outs.append(o.reshape(NS * C, OUT, OUT))
    return np.concatenate(outs, axis=0)


_PROG = None


def kernel(rois, h, w):
    global _PROG
    if _PROG is None:
        _PROG = build_program()
    in_maps = make_in_maps(rois, h, w)
    res = run_bass_kernel_spmd(_PROG, in_maps, list(range(NCORES)))
    return unpack_out(res)
